# revision 15
# baseline (speedup 1.0000x reference)
"""Trainium2 Bass kernel for a post-LN multi-head-attention block.

Reference computation (B=4, S=2048, D=1024, 16 heads x 64):
    q,k,v = x @ W{q,k,v}.T ; attn = softmax(q k^T/8 + mask) ; o = attn v
    out = LayerNorm(query + (o @ Wo.T)) * gamma + beta

Sharding: 8 cores = 4 batches x 2 query-halves (1024 query rows per core).
Each core computes all 16 heads for its query rows against the full
(mask-compacted) key set of its batch.  No collectives.

Key implementation choices:
  - keys with mask==0 contribute exactly 0 attention weight (additive -1e8
    underflows exp in f32), so the host compacts key/value to the unmasked
    subset, padded to a multiple of 128 (padding biased -1e30 so exp -> 0).
  - Q/K/V projections run in fp8e4m3 with DoubleRow perf mode (256-deep
    contraction per matmul, 2x PE throughput).  Host pre-scales the Q/K/V
    weights by 32 so their N(0,1/1024) entries stay out of the fp8
    subnormal range; the two 32x factors cancel in the softmax exp scale
    (0.125/1024) and V's 32x is divided out when draining PSUM into vaug.
  - scores are computed transposed, scoresT[k, q], in f32r (the 64-deep
    head contraction gains nothing from fp8), so softmax's k-reduction
    becomes a matmul reduction: V is augmented with a ones-column and
    attnV produces [out^T ; rowsum] in one PSUM accumulation group.
  - exp/scale/mask fold into one ScalarE activation per tile:
    E = exp(0.125/1024 * scoresT + maskbias[k] - 4), output fp8e4m3.
    The -4 shift keeps max(E) well under fp8e4m3's 240 max and
    cancels between softmax numerator and denominator.
  - attnV runs in fp8 DoubleRow over key-tile pairs (vaug fp8, E fp8).
  - normalization (divide by rowsum) happens after attnV via a K=1
    broadcast matmul of 1/rowsum and an elementwise multiply.
  - K^T/Q^T projections for head-pair j+1 are interleaved into pair j's
    (ScalarE-bound) attention loop so the TensorE never starves.
"""

import numpy as np
import ml_dtypes

import concourse.bacc as bacc
import concourse.tile as tile
import concourse.bass as bass
from concourse import mybir
from concourse.bass_utils import run_bass_kernel_spmd

DMODEL = 1024
NHEAD = 16
HD = 64
B = 4
S = 2048
NCORES = 8
SQ = 1024          # query rows per core
P = 128
F32 = mybir.dt.float32
F32R = mybir.dt.float32r
FP8 = mybir.dt.float8e4
NP_FP8 = ml_dtypes.float8_e4m3
ET = DMODEL // P   # 8 e-tiles (feature tiles)
DTL = DMODEL // P  # 8 d-tiles (contraction tiles)
NDR = DTL // 2     # 4 DoubleRow groups over the contraction
NQC = SQ // 512    # 2 query chunks of 512
NEC = DMODEL // 512  # 2 feature chunks of 512
NPAIR = NHEAD // 2   # 8 head pairs; pair j = heads (2j, 2j+1) in e-tile j
WSCALE = 32.0      # host premultiplier on Wq/Wk/Wv before fp8 cast
ESHIFT = -4.0      # exp bias shift: keeps max(E) under fp8e4m3 range


def _balanced_chunks(total, maxw=512):
    """Split `total` (a multiple of 128) into ~equal chunks <= maxw,
    each a multiple of 128."""
    nt = total // P
    nch = -(-total // maxw)
    base, rem = divmod(nt, nch)
    out, lo = [], 0
    for i in range(nch):
        w = (base + (1 if i < rem else 0)) * P
        out.append((lo, lo + w))
        lo += w
    return out


def _build(LPAD, do_compile=True, reps=1, phases=5, trivial_ln=True):
    KT = LPAD // P
    KCH = _balanced_chunks(LPAD)
    QCH = _balanced_chunks(SQ)
    BUFS = dict(av_sb=2, wkj=2, wqj=2, kts=2, qts=2, vh=512)
    nc = bacc.Bacc("TRN2", target_bir_lowering=False, debug=False,
                   num_devices=NCORES)

    qT = nc.declare_dram_parameter("qT", [DMODEL, SQ], FP8, isOutput=False)
    kT = nc.declare_dram_parameter("kT", [DMODEL, LPAD], FP8, isOutput=False)
    vT = nc.declare_dram_parameter("vT", [DMODEL, LPAD], FP8, isOutput=False)
    resid = nc.declare_dram_parameter("resid", [SQ, DMODEL], F32, isOutput=False)
    wqT = nc.declare_dram_parameter("wqT", [DMODEL, DMODEL], FP8, isOutput=False)
    wkT = nc.declare_dram_parameter("wkT", [DMODEL, DMODEL], FP8, isOutput=False)
    wvT = nc.declare_dram_parameter("wvT", [DMODEL, DMODEL], FP8, isOutput=False)
    woT = nc.declare_dram_parameter("woT", [DMODEL, DMODEL], F32R, isOutput=False)
    maskb = nc.declare_dram_parameter("maskb", [P, KT], F32, isOutput=False)
    gamma = nc.declare_dram_parameter("gamma", [DMODEL], F32, isOutput=False)
    beta = nc.declare_dram_parameter("beta", [DMODEL], F32, isOutput=False)
    out = nc.declare_dram_parameter("out", [SQ, DMODEL], F32, isOutput=True)

    def dram3(ap):
        # (o*P, width) DRAM tensor viewed as [p, o, width]
        return ap.rearrange("(o p) w -> p o w", p=P)

    with tile.TileContext(nc) as tc:
        with (
            tc.tile_pool(name="keep", bufs=1) as keep,      # long-lived SBUF
            tc.tile_pool(name="wpool", bufs=1) as wpool,    # weights (phased)
            tc.tile_pool(name="pproj", bufs=2, space="PSUM") as pproj,
            tc.tile_pool(name="pattn", bufs=1, space="PSUM") as pattn,
        ):
            for _rep in range(reps):
                self_body(nc, tc, keep, wpool, pproj, pattn, phases,
                          BUFS, LPAD, KT, KCH, QCH,
                          qT, kT, vT, resid, wqT, wkT, wvT, woT,
                          maskb, gamma, beta, out, dram3, trivial_ln)
    if do_compile:
        nc.compile()
    return nc


def self_body(nc, tc, keep, wpool, pproj, pattn, phases, BUFS, LPAD, KT,
              KCH, QCH,
              qT, kT, vT, resid, wqT, wkT, wvT, woT, maskb, gamma, beta,
              out, dram3, trivial_ln=True):
    NKP = KT // 2        # DoubleRow key-tile pairs in attnV
    KODD = KT % 2 == 1   # trailing single key tile
    # ---- long-lived tensors ----
    vaug = keep.tile([P, KT, NHEAD, HD + 1], FP8)    # [k | head | V,1]
    aoT = keep.tile([P, ET, SQ], F32R)               # attn out^T (d' on part)
    maskb_sb = keep.tile([P, KT], F32)
    nc.gpsimd.dma_start(out=maskb_sb, in_=maskb.ap())
    ones_sb = keep.tile([65, 64], F32R)
    nc.vector.memset(ones_sb[64:65, :].bitcast(F32), 1.0)
    for kt in range(KT):
        nc.vector.memset(vaug[:, kt, :, HD:HD + 1], 1.0)

    with (
        tc.tile_pool(name="din", bufs=1) as din,
        tc.tile_pool(name="dpool", bufs=2) as dpool,
    ):
        # resident contraction inputs for the K^T / Q^T projections
        # (DMA'd after phase A's inputs so V-projection starts immediately)
        kin = din.tile([P, DTL, LPAD], FP8)
        qin = din.tile([P, DTL, SQ], FP8)

        vT3 = dram3(vT.ap())
        wvT3 = dram3(wvT.ap())
        kT3 = dram3(kT.ap())
        qT3 = dram3(qT.ap())
        wkT3 = dram3(wkT.ap())
        wqT3 = dram3(wqT.ap())
        epool = tc.alloc_tile_pool(name="epool", bufs=2)
        wvpool = tc.alloc_tile_pool(name="wvpool", bufs=1)

        VH = BUFS["vh"]
        NVP = DMODEL // VH       # V-projection weight slices
        HPS = VH // HD           # heads per slice

        def vproj_half(ec):
            """Returns trace thunks for one feature-slice of the V
            projection (heads HPS*ec ...).  Weight-slice DMA traced now."""
            wv_h = wvpool.tile([P, DTL, VH], FP8, tag="wv")
            nc.sync.dma_start(out=wv_h,
                              in_=wvT3[:, :, ec * VH:(ec + 1) * VH])
            thunks = []
            for kt in range(KT):
                def vthunk(kt=kt, ec=ec, wv_h=wv_h):
                    vin = dpool.tile([P, DTL, P], FP8, tag="vin", bufs=4)
                    nc.gpsimd.dma_start(out=vin,
                                        in_=vT3[:, :, kt * P:(kt + 1) * P])
                    ps = pproj.tile([P, 512], F32, tag="pp")
                    for dt in range(NDR):
                        nc.tensor.matmul(
                            ps[:, :VH],
                            lhsT=vin[:, 2 * dt:2 * dt + 2, :],
                            rhs=wv_h[:, 2 * dt:2 * dt + 2, :],
                            start=(dt == 0), stop=(dt == NDR - 1),
                            perf_mode=mybir.MatmulPerfMode.DoubleRow)
                    nc.vector.tensor_scalar(
                        out=vaug[:, kt, ec * HPS:(ec + 1) * HPS, 0:HD],
                        in0=ps[:, :VH].rearrange("p (h x) -> p h x", x=HD),
                        scalar1=1.0 / WSCALE, scalar2=None,
                        op0=mybir.AluOpType.mult)
                thunks.append(vthunk)
            return thunks

        def project_pair(j):
            """Trace K^T and Q^T projection for head-pair j (streamed
            per-pair weight column slices).  Returns (kts, qts) tiles plus
            deferred trace thunks (one PSUM group = 4 DR matmuls + copy)."""
            wkj = dpool.tile([P, DTL, P], FP8, tag="wkj", bufs=BUFS["wkj"])
            nc.sync.dma_start(out=wkj, in_=wkT3[:, :, j * P:(j + 1) * P])
            wqj = dpool.tile([P, DTL, P], FP8, tag="wqj", bufs=BUFS["wqj"])
            nc.sync.dma_start(out=wqj, in_=wqT3[:, :, j * P:(j + 1) * P])
            kts = dpool.tile([P, LPAD], F32R, tag="kts", bufs=BUFS["kts"])
            qts = dpool.tile([P, SQ], F32R, tag="qts", bufs=BUFS["qts"])
            thunks = []
            for (lo, hi) in KCH:
                def kthunk(lo=lo, hi=hi, kts=kts, wkj=wkj):
                    w = hi - lo
                    ps = pproj.tile([P, 512], F32, tag="pp")
                    for dt in range(NDR):
                        nc.tensor.matmul(
                            ps[:, :w],
                            lhsT=wkj[:, 2 * dt:2 * dt + 2, :],
                            rhs=kin[:, 2 * dt:2 * dt + 2, lo:hi],
                            start=(dt == 0), stop=(dt == NDR - 1),
                            perf_mode=mybir.MatmulPerfMode.DoubleRow)
                    nc.vector.tensor_copy(out=kts[:, lo:hi], in_=ps[:, :w])
                thunks.append(kthunk)
            for (lo, hi) in QCH:
                def qthunk(lo=lo, hi=hi, qts=qts, wqj=wqj):
                    w = hi - lo
                    ps = pproj.tile([P, 512], F32, tag="pp")
                    for dt in range(NDR):
                        nc.tensor.matmul(
                            ps[:, :w],
                            lhsT=wqj[:, 2 * dt:2 * dt + 2, :],
                            rhs=qin[:, 2 * dt:2 * dt + 2, lo:hi],
                            start=(dt == 0), stop=(dt == NDR - 1),
                            perf_mode=mybir.MatmulPerfMode.DoubleRow)
                    nc.vector.tensor_copy(out=qts[:, lo:hi], in_=ps[:, :w])
                thunks.append(qthunk)
            return kts, qts, thunks

        if phases < 4:
            # projections only (for phase bisection)
            for ec in range(NVP):
                for t in vproj_half(ec):
                    t()
            wvpool.release()
            for j in range(NPAIR):
                kts, qts, thunks = project_pair(j)
                for t in thunks:
                    t()
                nc.sync.dma_start(out=out.ap()[0:P, 0:LPAD],
                                  in_=kts.bitcast(F32))
                nc.sync.dma_start(out=out.ap()[P:2 * P, 0:SQ],
                                  in_=qts.bitcast(F32))
            return

        # ======== prologue: pair 0's projections + first V half ========
        # DMA order = need order: pair-0 proj weights (SP queue) while
        # kin/qin stream in parallel on the Pool-triggered queue.
        kts, qts, thunks = project_pair(0)
        for (lo, hi) in KCH:
            nc.gpsimd.dma_start(out=kin[:, :, lo:hi], in_=kT3[:, :, lo:hi])
        for (lo, hi) in QCH:
            nc.gpsimd.dma_start(out=qin[:, :, lo:hi], in_=qT3[:, :, lo:hi])
        vthunks0 = vproj_half(0)
        for t in thunks:          # pair-0 K^T/Q^T projections
            t()
        for t in vthunks0:        # V projection for heads 0..7
            t()

        # phase-E tiles are prefetched during the attention loop
        resid3 = dram3(resid.ap())
        out3 = dram3(out.ap())
        estate = {}

        def prefetch_wo():
            wo_sb = epool.tile([P, DTL, DMODEL], F32R, tag="wo", bufs=1)
            nc.gpsimd.dma_start(out=wo_sb, in_=dram3(woT.ap()))
            estate.update(wo_sb=wo_sb)
            if not trivial_ln:
                gamma_sb = epool.tile([P, DMODEL], F32, tag="gamma", bufs=1)
                nc.gpsimd.dma_start(out=gamma_sb, in_=bass.AP(
                    tensor=gamma.ap().tensor, offset=0,
                    ap=[[0, P], [1, DMODEL]]))
                beta_sb = epool.tile([P, DMODEL], F32, tag="beta", bufs=1)
                nc.gpsimd.dma_start(out=beta_sb, in_=bass.AP(
                    tensor=beta.ap().tensor, offset=0,
                    ap=[[0, P], [1, DMODEL]]))
                estate.update(gamma_sb=gamma_sb, beta_sb=beta_sb)
            eps_sb = epool.tile([P, 1], F32, tag="eps", bufs=1)
            nc.vector.memset(eps_sb, 1e-5)
            estate.update(eps_sb=eps_sb)

        def prefetch_rin():
            rins = []
            for st in range(SQ // P):
                rin = epool.tile([P, DMODEL], F32, tag="rin", bufs=8)
                nc.gpsimd.dma_start(out=rin, in_=resid3[:, st, :])
                rins.append(rin)
            estate["rins"] = rins

        def emit_ln(st):
            """Output projection + residual + layernorm for one 128-row
            query tile (needs aoT complete for those rows)."""
            wo_sb = estate["wo_sb"]
            x_t = epool.tile([P, DMODEL], F32, tag="x", bufs=3)
            for ec in range(NEC):
                ps = pproj.tile([P, 512], F32, tag="pp")
                for dj in range(DTL):
                    nc.tensor.matmul(
                        ps,
                        lhsT=aoT[:, dj, st * P:(st + 1) * P],
                        rhs=wo_sb[:, dj, ec * 512:(ec + 1) * 512],
                        start=(dj == 0), stop=(dj == DTL - 1))
                nc.vector.tensor_add(
                    out=x_t[:, ec * 512:(ec + 1) * 512],
                    in0=ps,
                    in1=estate["rins"][st][:, ec * 512:(ec + 1) * 512])
            stats = epool.tile([P, 2, 6], F32, tag="stats")
            nc.vector.bn_stats(out=stats[:, 0, :], in_=x_t[:, 0:512])
            nc.vector.bn_stats(out=stats[:, 1, :], in_=x_t[:, 512:1024])
            mv = epool.tile([P, 2], F32, tag="mv")
            nc.vector.bn_aggr(out=mv, in_=stats)
            sd = epool.tile([P, 2], F32, tag="sd")
            nc.scalar.activation(out=sd[:, 0:1], in_=mv[:, 1:2],
                                 func=mybir.ActivationFunctionType.Sqrt,
                                 bias=estate["eps_sb"][:, 0:1], scale=1.0)
            nc.vector.reciprocal(out=sd[:, 1:2], in_=sd[:, 0:1])
            nc.vector.tensor_scalar(
                out=x_t, in0=x_t, scalar1=mv[:, 0:1], scalar2=sd[:, 1:2],
                op0=mybir.AluOpType.subtract, op1=mybir.AluOpType.mult)
            if not trivial_ln:
                nc.gpsimd.tensor_mul(out=x_t, in0=x_t,
                                     in1=estate["gamma_sb"])
                nc.gpsimd.tensor_add(out=x_t, in0=x_t,
                                     in1=estate["beta_sb"])
            nc.sync.dma_start(out=out3[:, st, :], in_=x_t)

        # ======== phase D: attention; pair j+1's projections and the
        # V-projection halves run as TensorE filler work inside the loop;
        # each (j, qc) epilogue is deferred past the next iteration's
        # first scores so ScalarE never waits at iteration boundaries ====
        pend = [None]
        for j in range(NPAIR):
            if j == 1 and phases >= 5:
                prefetch_wo()               # wo/gamma/beta stream on Pool
            if j == NPAIR - 2 and phases >= 5:
                prefetch_rin()              # residual rows stream on Pool
            fill = []
            if 0 <= j < NVP - 1:
                fill += vproj_half(j + 1)   # next V-projection slice
            if j + 1 < NPAIR:
                kts_n, qts_n, pf = project_pair(j + 1)
                fill += pf
            else:
                kts_n = qts_n = None

            for qc in range(NQC):
                qsl = slice(qc * 512, (qc + 1) * 512)
                avA = pattn.tile([HD + 1, 512], F32, tag="avA")
                avB = pattn.tile([HD + 1, 512], F32, tag="avB")
                es = []

                def scores(kt):
                    ps = pproj.tile([P, 1024], F32, tag="ps_s", bufs=2)
                    nc.tensor.matmul(
                        ps[:, 0:512],
                        lhsT=kts[0:64, kt * P:(kt + 1) * P],
                        rhs=qts[0:64, qsl], start=True, stop=True)
                    nc.tensor.matmul(
                        ps[:, 512:1024],
                        lhsT=kts[64:128, kt * P:(kt + 1) * P],
                        rhs=qts[64:128, qsl], start=True, stop=True)
                    # pair-slot fp8 exp output for DoubleRow attnV
                    if kt % 2 == 0:
                        e2 = dpool.tile([P, 2, 1024], FP8, tag="e2", bufs=4)
                        es.append(e2)
                    else:
                        e2 = es[kt // 2]
                    nc.scalar.activation(
                        out=e2[:, kt % 2, :], in_=ps,
                        func=mybir.ActivationFunctionType.Exp,
                        bias=maskb_sb[:, kt:kt + 1],
                        scale=0.125 / (WSCALE * WSCALE))

                def attnvp(t, es=es, avA=avA, avB=avB, j=j):
                    # DoubleRow over key tiles (2t, 2t+1)
                    e2 = es[t]
                    nc.tensor.matmul(
                        avA, lhsT=vaug[:, 2 * t:2 * t + 2, 2 * j, :],
                        rhs=e2[:, :, 0:512],
                        start=(t == 0), stop=False,
                        perf_mode=mybir.MatmulPerfMode.DoubleRow)
                    nc.tensor.matmul(
                        avB, lhsT=vaug[:, 2 * t:2 * t + 2, 2 * j + 1, :],
                        rhs=e2[:, :, 512:1024],
                        start=(t == 0), stop=False,
                        perf_mode=mybir.MatmulPerfMode.DoubleRow)

                def attnv_last(es=es, avA=avA, avB=avB, j=j):
                    # trailing single key tile (KT odd) closes the group
                    e2 = es[KT // 2]
                    nc.tensor.matmul(
                        avA, lhsT=vaug[:, KT - 1, 2 * j, :],
                        rhs=e2[:, 0, 0:512],
                        start=False, stop=True)
                    nc.tensor.matmul(
                        avB, lhsT=vaug[:, KT - 1, 2 * j + 1, :],
                        rhs=e2[:, 0, 512:1024],
                        start=False, stop=True)

                # software pipeline: scores(kt) ahead of attnvp(t-1);
                # previous iteration's epilogue lands after scores(0)
                scores(0)
                if pend[0] is not None:
                    pend[0]()          # prev iteration's attnv tail + epilogue
                    pend[0] = None
                scores(1)
                for t in range(1, NKP):
                    scores(2 * t)
                    attnvp(t - 1)
                    if fill:
                        fill.pop(0)()
                    scores(2 * t + 1)
                    if fill:
                        fill.pop(0)()
                if KODD:
                    scores(KT - 1)
                attnvp(NKP - 1)
                if fill:
                    fill.pop(0)()
                if fill and qc == NQC - 1:
                    while fill:
                        fill.pop(0)()
                if j == NPAIR - 1 and qc == 1 and phases >= 5:
                    # first-half LN tiles need only qc=0 epilogues (all
                    # retired by the pend() above); their out-proj matmuls
                    # keep PE busy while ScalarE finishes the last exps
                    for st in range(4):
                        emit_ln(st)

                def epilogue(avA=avA, avB=avB, j=j, qsl=qsl,
                             attnv_last=attnv_last):
                    attnv_last()       # deferred pipeline tail
                    # drain the PSUM accumulators to SBUF right away so
                    # the banks free up for the next iteration's attnV
                    av_sb = dpool.tile([65, 1024], F32R, tag="av_sb",
                                       bufs=BUFS["av_sb"])
                    nc.vector.tensor_copy(out=av_sb[:, 0:512], in_=avA)
                    nc.vector.tensor_copy(out=av_sb[:, 512:1024], in_=avB)

                    # normalize: aoT = av / rowsum
                    recip_t = dpool.tile([65, 1024], F32R, tag="recip",
                                         bufs=1)
                    with nc.allow_low_precision(
                            reason="f32r recip feeds f32r matmul"):
                        nc.vector.reciprocal(
                            out=recip_t[64:65, :],
                            in_=av_sb[64:65, :].bitcast(F32))
                    # reuse the (drained) attention accumulator banks so
                    # the rb matmuls never block projection-filler PSUM
                    rbA = pattn.tile([64, 512], F32, tag="avA")
                    rbB = pattn.tile([64, 512], F32, tag="avB")
                    nc.tensor.matmul(rbA, lhsT=ones_sb[64:65, :],
                                     rhs=recip_t[64:65, 0:512],
                                     start=True, stop=True)
                    nc.tensor.matmul(rbB, lhsT=ones_sb[64:65, :],
                                     rhs=recip_t[64:65, 512:1024],
                                     start=True, stop=True)
                    # normalize multiplies read the broadcast rows
                    # straight from PSUM
                    nc.vector.tensor_tensor(
                        aoT[0:64, j, qsl], av_sb[0:64, 0:512].bitcast(F32),
                        rbA, mybir.AluOpType.mult)
                    nc.vector.tensor_tensor(
                        av_sb[0:64, 512:1024],
                        av_sb[0:64, 512:1024].bitcast(F32),
                        rbB, mybir.AluOpType.mult)
                    nc.sync.dma_start(out=aoT[64:128, j, qsl],
                                      in_=av_sb[0:64, 512:1024])

                pend[0] = epilogue

            kts, qts = kts_n, qts_n
        if pend[0] is not None:
            pend[0]()
            pend[0] = None
        wvpool.release()

        if phases < 5:
            with tc.tile_pool(name="dump", bufs=1) as dump:
                t = dump.tile([P, 512], F32)
                nc.vector.tensor_copy(out=t,
                                      in_=aoT[:, 0, 0:512].bitcast(F32))
                nc.sync.dma_start(out=out.ap()[0:P, 0:512], in_=t)
            epool.release()
            return

        # ======== phase E tail: LN for the second query half ========
        for st in range(4, SQ // P):
            emit_ln(st)
        epool.release()


_cache = {}


def _get_nc(LPAD, trivial_ln=True):
    key = (LPAD, trivial_ln)
    if key not in _cache:
        _cache[key] = _build(LPAD, trivial_ln=trivial_ln)
    return _cache[key]


def make_in_maps(query, key, value, mask, Wq, Wk, Wv, Wo, ln_gamma, ln_beta):
    """Host-side sharding: returns (in_maps, LPAD)."""
    f = lambda a: np.ascontiguousarray(np.asarray(a, np.float32))
    f8 = lambda a: np.ascontiguousarray(np.asarray(a, np.float32)).astype(NP_FP8)
    query, key, value = f(query), f(key), f(value)
    mask = np.asarray(mask)
    wqT = f8(np.asarray(Wq, np.float32).T * WSCALE)
    wkT = f8(np.asarray(Wk, np.float32).T * WSCALE)
    wvT = f8(np.asarray(Wv, np.float32).T * WSCALE)
    woT = f(np.asarray(Wo, np.float32).T)
    gamma, beta = f(ln_gamma), f(ln_beta)

    idxs = []
    for b in range(B):
        ix = np.nonzero(mask[b] != 0)[0]
        if len(ix) == 0:
            # all-masked row: the -1e8 bias is common to every key, so the
            # reference softmax reduces to plain softmax over all keys.
            ix = np.arange(S)
        idxs.append(ix)
    Lmax = max(len(ix) for ix in idxs)
    LPAD = max(P, ((Lmax + P - 1) // P) * P)
    KT = LPAD // P

    in_maps = []
    for c in range(NCORES):
        b, g = divmod(c, 2)
        ix = idxs[b]
        L = len(ix)
        kc = np.zeros((LPAD, DMODEL), np.float32)
        kc[:L] = key[b][ix]
        vc = np.zeros((LPAD, DMODEL), np.float32)
        vc[:L] = value[b][ix]
        mb = np.full((LPAD,), -1e30, np.float32)
        mb[:L] = ESHIFT
        qrows = query[b, g * SQ:(g + 1) * SQ]
        in_maps.append({
            "qT": np.ascontiguousarray(qrows.T).astype(NP_FP8),
            "kT": np.ascontiguousarray(kc.T).astype(NP_FP8),
            "vT": np.ascontiguousarray(vc.T).astype(NP_FP8),
            "resid": np.ascontiguousarray(qrows),
            "wqT": wqT, "wkT": wkT, "wvT": wvT, "woT": woT,
            "maskb": np.ascontiguousarray(mb.reshape(KT, P).T),
            "gamma": gamma, "beta": beta,
        })
    return in_maps, LPAD


def gather_out(results):
    out = np.empty((B, S, DMODEL), np.float32)
    for c in range(NCORES):
        b, g = divmod(c, 2)
        out[b, g * SQ:(g + 1) * SQ] = results[c]["out"]
    return out


def kernel(query, key, value, mask, Wq, Wk, Wv, Wo, ln_gamma, ln_beta):
    in_maps, LPAD = make_in_maps(query, key, value, mask, Wq, Wk, Wv, Wo,
                                 ln_gamma, ln_beta)
    g = np.asarray(ln_gamma, np.float32)
    bt = np.asarray(ln_beta, np.float32)
    trivial_ln = bool(np.all(g == 1.0) and np.all(bt == 0.0))
    nc = _get_nc(LPAD, trivial_ln)
    res = run_bass_kernel_spmd(nc, in_maps, list(range(NCORES)))
    return gather_out(res.results)
